# revision 7
# baseline (speedup 1.0000x reference)
"""Trainium2 Bass kernel for LocalSelfAttentionUnFold.

Reference math (B=4, S=2048, E=256, H=8, D=32, W=33, pad=16, K=S-W+1=2016):
  q,k,v = x @ W* + b*            -> [B,S,E] -> heads [B,H,S,D]
  scores[b,h,s,kx] = sum_{w,d} q_pad[b,h,s+w,d] * k[b,h,kx+w,d] * D^-0.5
  attn = softmax(scores, axis=kx)             # dense [S, K] matrix!
  out[b,h,s,d]  = sum_{kx} attn[s,kx] * vsum[kx,d],  vsum[kx] = sum_w v[kx+w]

Kernel strategy (per NeuronCore; 8 cores, core c handles batch b=c//2 and
head group hg=c%2, i.e. 4 heads = 128 embedding columns):
  - scores as a dense GEMM with the (w,d)-flattened contraction of 1056,
    done as 9 PSUM-accumulated matmuls of contraction 128 (last 32).
    Operands are "4-fold shifted" copies of q^T / k^T (Q4s / K4s) so each
    128-chunk of the contraction is a plain free-dim slice.
  - softmax row-wise (q on partitions): DVE max, ACT exp (+accum rowsum).
  - attn transposed per 128-chunk on the tensor engine, then
    out[q,d] = sum_c attnT[c].T @ vsum[c] accumulated in PSUM.
  - vsum via log-doubling shifted adds on DVE (all 4 heads at once).
All matmul operands fp16 (measured end-to-end rel err ~1.4e-3), PSUM f32.
"""

import numpy as np
from contextlib import ExitStack

S = 2048
E = 256
D = 32
WIN = 33
PAD = 16
K = S - WIN + 1  # 2016
NHPC = 4  # heads per core
SCALE = float(D) ** -0.5
NCORES = 8

_CACHE: dict = {}


def _build_nc():
    import concourse.bass as bass
    import concourse.tile as tile
    from concourse import bacc, mybir
    from concourse.masks import make_identity

    fp16 = mybir.dt.float16
    f32 = mybir.dt.float32
    AF = mybir.ActivationFunctionType
    AX = mybir.AxisListType

    nc = bacc.Bacc("TRN2", target_bir_lowering=False, debug=False,
                   num_devices=NCORES)

    xT_d = nc.dram_tensor("xT", [E, S], f32, kind="ExternalInput").ap()
    wq_d = nc.dram_tensor("wq", [E, 128], f32, kind="ExternalInput").ap()
    wk_d = nc.dram_tensor("wk", [E, 128], f32, kind="ExternalInput").ap()
    wv_d = nc.dram_tensor("wv", [E, 128], f32, kind="ExternalInput").ap()
    bqs_d = nc.dram_tensor("bqs", [128, 1], f32, kind="ExternalInput").ap()
    bk_d = nc.dram_tensor("bk", [128, 1], f32, kind="ExternalInput").ap()
    bv_d = nc.dram_tensor("bv", [128, 1], f32, kind="ExternalInput").ap()
    out_d = nc.dram_tensor("out", [S, 128], f32, kind="ExternalOutput").ap()

    with tile.TileContext(nc) as tc, ExitStack() as ctx:
        const = ctx.enter_context(tc.tile_pool(name="const", bufs=1))
        persist = ctx.enter_context(tc.tile_pool(name="persist", bufs=1))

        ident = const.tile([128, 128], fp16)
        make_identity(nc, ident[:])

        # ---- load + cast inputs ----
        x16 = persist.tile([128, 2, S], fp16)  # x16[:, i, :] = xT[128i:128i+128, :]
        w16 = {}
        biases = {}
        with tc.tile_pool(name="ldtmp", bufs=2) as ldtmp:
            for i in range(2):
                xf = ldtmp.tile([128, S], f32, tag="xf")
                nc.gpsimd.dma_start(out=xf[:], in_=xT_d[i * 128:(i + 1) * 128, :])
                nc.vector.tensor_copy(out=x16[:, i, :], in_=xf[:])
            for name, wd in (("q", wq_d), ("k", wk_d), ("v", wv_d)):
                wt = const.tile([128, 2, 128], fp16, tag=f"w{name}")
                for i in range(2):
                    wf = ldtmp.tile([128, 128], f32, tag="wf")
                    nc.gpsimd.dma_start(out=wf[:], in_=wd[i * 128:(i + 1) * 128, :])
                    nc.vector.tensor_copy(out=wt[:, i, :], in_=wf[:])
                w16[name] = wt
            for name, bd in (("q", bqs_d), ("k", bk_d), ("v", bv_d)):
                bt = const.tile([128, 1], f32, tag=f"b{name}")
                nc.gpsimd.dma_start(out=bt[:], in_=bd[:, :])
                biases[name] = bt

        # ---- projections: q^T,k^T,v^T [128, S] fp16 (q pre-scaled) ----
        qkv16 = {}
        with tc.tile_pool(name="pproj", bufs=2, space="PSUM") as pproj:
            for name in ("q", "k", "v"):
                dst = persist.tile([128, S], fp16, tag=f"{name}16T")
                qkv16[name] = dst
                sc = SCALE if name == "q" else 1.0
                for sb in range(4):
                    ps = pproj.tile([128, 512], f32, tag="pp")
                    nc.tensor.matmul(ps[:], lhsT=w16[name][:, 0, :],
                                     rhs=x16[:, 0, sb * 512:(sb + 1) * 512],
                                     start=True, stop=False)
                    nc.tensor.matmul(ps[:], lhsT=w16[name][:, 1, :],
                                     rhs=x16[:, 1, sb * 512:(sb + 1) * 512],
                                     start=False, stop=True)
                    nc.scalar.activation(out=dst[:, sb * 512:(sb + 1) * 512],
                                         in_=ps[:], func=AF.Identity,
                                         bias=biases[name], scale=sc)
        q16T, k16T, v16T = qkv16["q"], qkv16["k"], qkv16["v"]

        # ---- vsum^T[128, K] fp16 via log-doubling box filter (all heads) ----
        vsumT = persist.tile([128, K], fp16)
        with tc.tile_pool(name="dbl", bufs=2) as dblp:
            t2 = dblp.tile([128, 2047], f32, tag="dbl")
            nc.vector.tensor_add(t2[:], v16T[:, 0:2047], v16T[:, 1:2048])
            prev, plen = t2, 2047
            for wshift in (2, 4, 8, 16):
                cur_len = plen - wshift
                cur = dblp.tile([128, 2045], f32, tag="dbl")
                nc.vector.tensor_add(cur[:, 0:cur_len], prev[:, 0:cur_len],
                                     prev[:, wshift:wshift + cur_len])
                prev, plen = cur, cur_len
            # width-32 sums now in prev[:, 0:2017]; add v[j+32] -> width 33
            nc.vector.tensor_add(vsumT[:], prev[:, 0:K], v16T[:, 32:32 + K])

        # ---- pools for the main loop ----
        kq = ctx.enter_context(tc.tile_pool(name="kq", bufs=2))
        vs = ctx.enter_context(tc.tile_pool(name="vs", bufs=2))
        apool = ctx.enter_context(tc.tile_pool(name="apool", bufs=3))
        atpool = ctx.enter_context(tc.tile_pool(name="atpool", bufs=2))
        stats = ctx.enter_context(tc.tile_pool(name="stats", bufs=4))
        opool = ctx.enter_context(tc.tile_pool(name="opool", bufs=4))
        psum_sc = ctx.enter_context(tc.tile_pool(name="psc", bufs=2, space="PSUM"))
        psum_t = ctx.enter_context(tc.tile_pool(name="pst", bufs=2, space="PSUM"))
        psum_o = ctx.enter_context(tc.tile_pool(name="pso", bufs=2, space="PSUM"))

        for h in range(NHPC):
            hp = 32 * h  # head's partition offset in q/k/v^T

            # vsum chunks [kx 128, d 32] via PE transpose of vsumT rows
            vsum_sb = vs.tile([128, 16, D], fp16, tag="vsum")
            for ch in range(16):
                cc = 128 if ch < 15 else K - 15 * 128  # 96
                pt = psum_t.tile([128, 128], fp16, tag="ptr")
                nc.tensor.transpose(out=pt[0:cc, 0:D],
                                    in_=vsumT[hp:hp + 32, ch * 128:ch * 128 + cc],
                                    identity=ident[hp:hp + 32, hp:hp + 32],
                                    tile_position=(hp, 0))
                nc.vector.tensor_copy(out=vsum_sb[0:cc, ch, :], in_=pt[0:cc, 0:D])

            # K4s[32r+d, j] = k^T[hp+d, j+r];  Q4s[32r+d, i] = q_pad^T[hp+d, i+r]
            K4s = kq.tile([128, S], fp16, tag="k4s")
            for r in range(4):
                nc.gpsimd.dma_start(out=K4s[32 * r:32 * r + 32, 0:S - r],
                                  in_=k16T[hp:hp + 32, r:S])
            Q4s = kq.tile([128, S + 2 * PAD], fp16, tag="q4s")
            nc.vector.memset(Q4s[:, 0:PAD], 0.0)
            nc.vector.memset(Q4s[:, S:S + 2 * PAD], 0.0)
            for r in range(4):
                nc.gpsimd.dma_start(out=Q4s[32 * r:32 * r + 32, PAD - r:PAD - r + S],
                                  in_=q16T[hp:hp + 32, 0:S])

            for t in range(16):
                q0 = t * 128
                halves = []
                for half in range(2):
                    ps = psum_sc.tile([128, 2, 512], f32, tag="scores")
                    for blk in range(2):
                        c0 = (half * 2 + blk) * 504
                        for a in range(8):
                            nc.tensor.matmul(
                                ps[:, blk, 0:504],
                                lhsT=Q4s[:, q0 + 4 * a:q0 + 4 * a + 128],
                                rhs=K4s[:, 4 * a + c0:4 * a + c0 + 504],
                                start=(a == 0), stop=False)
                        nc.tensor.matmul(
                            ps[:, blk, 0:504],
                            lhsT=Q4s[0:32, q0 + 32:q0 + 160],
                            rhs=K4s[0:32, 32 + c0:32 + c0 + 504],
                            start=False, stop=True)
                    halves.append(ps)

                mx2 = stats.tile([128, 2], f32, tag="mx2")
                for half in range(2):
                    nc.vector.tensor_reduce(out=mx2[:, half:half + 1],
                                            in_=halves[half][:, :, 0:504],
                                            op=mybir.AluOpType.max, axis=AX.XY)
                negmx = stats.tile([128, 1], f32, tag="negmx")
                nc.vector.tensor_reduce(out=negmx[:], in_=mx2[:],
                                        op=mybir.AluOpType.max, axis=AX.X,
                                        negate=True)

                attn = apool.tile([128, K], fp16, tag="attn")
                racc = stats.tile([128, 4], f32, tag="racc")
                for half in range(2):
                    for blk in range(2):
                        j = half * 2 + blk
                        nc.scalar.activation(
                            out=attn[:, j * 504:(j + 1) * 504],
                            in_=halves[half][:, blk, 0:504],
                            func=AF.Exp, bias=negmx[:], scale=1.0,
                            accum_out=racc[:, j:j + 1])
                rsum = stats.tile([128, 1], f32, tag="rsum")
                nc.vector.tensor_reduce(out=rsum[:], in_=racc[:],
                                        op=mybir.AluOpType.add, axis=AX.X)
                rinv = stats.tile([128, 1], f32, tag="rinv")
                nc.vector.reciprocal(out=rinv[:], in_=rsum[:])

                attnT = atpool.tile([128, 16, 128], fp16, tag="attnT")
                for ch in range(16):
                    cc = 128 if ch < 15 else K - 15 * 128
                    pt = psum_t.tile([128, 128], fp16, tag="ptr")
                    nc.tensor.transpose(out=pt[0:cc, :],
                                        in_=attn[:, ch * 128:ch * 128 + cc],
                                        identity=ident[:, :])
                    nc.vector.tensor_copy(out=attnT[0:cc, ch, :], in_=pt[0:cc, :])

                po = psum_o.tile([128, D], f32, tag="pav")
                for ch in range(16):
                    cc = 128 if ch < 15 else K - 15 * 128
                    nc.tensor.matmul(po[:], lhsT=attnT[0:cc, ch, :],
                                     rhs=vsum_sb[0:cc, ch, :],
                                     start=(ch == 0), stop=(ch == 15))
                ob = opool.tile([128, D], f32, tag="ob")
                nc.vector.tensor_scalar_mul(ob[:], po[:], rinv[:])
                nc.gpsimd.dma_start(out=out_d[q0:q0 + 128, hp:hp + 32], in_=ob[:])

    nc.compile()
    return nc


def _get_nc():
    if "nc" not in _CACHE:
        _CACHE["nc"] = _build_nc()
    return _CACHE["nc"]


def kernel(x, Wq, bq, Wk, bk, Wv, bv):
    from concourse.bass_utils import run_bass_kernel_spmd

    nc = _get_nc()
    x = np.asarray(x, dtype=np.float32)
    in_maps = []
    for c in range(NCORES):
        b, hg = c // 2, c % 2
        sl = slice(hg * 128, (hg + 1) * 128)
        in_maps.append({
            "xT": np.ascontiguousarray(x[b].T),
            "wq": np.ascontiguousarray(np.asarray(Wq, np.float32)[:, sl]),
            "wk": np.ascontiguousarray(np.asarray(Wk, np.float32)[:, sl]),
            "wv": np.ascontiguousarray(np.asarray(Wv, np.float32)[:, sl]),
            "bqs": np.ascontiguousarray(
                (np.asarray(bq, np.float32)[sl] * SCALE).reshape(128, 1)),
            "bk": np.ascontiguousarray(np.asarray(bk, np.float32)[sl].reshape(128, 1)),
            "bv": np.ascontiguousarray(np.asarray(bv, np.float32)[sl].reshape(128, 1)),
        })
    res = run_bass_kernel_spmd(nc, in_maps, list(range(NCORES)))
    out = np.empty((4, S, E), np.float32)
    for c in range(NCORES):
        b, hg = c // 2, c % 2
        out[b, :, hg * 128:(hg + 1) * 128] = res.results[c]["out"]
    return out


# revision 16
# speedup vs baseline: 1.0941x; 1.0941x over previous
"""Trainium2 Bass kernel for LocalSelfAttentionUnFold.

Reference math (B=4, S=2048, E=256, H=8, D=32, W=33, pad=16, K=S-W+1=2016):
  q,k,v = x @ W* + b*            -> [B,S,E] -> heads [B,H,S,D]
  scores[b,h,s,kx] = sum_{w,d} q_pad[b,h,s+w,d] * k[b,h,kx+w,d] * D^-0.5
  attn = softmax(scores, axis=kx)             # dense [S, K] matrix!
  out[b,h,s,d]  = sum_{kx} attn[s,kx] * vsum[kx,d],  vsum[kx] = sum_w v[kx+w]

Kernel strategy (per NeuronCore; 8 cores, core c handles batch b=c//2 and
head group hg=c%2, i.e. 4 heads = 128 embedding columns):
  - scores as a dense GEMM with the (w,d)-flattened contraction of 1056,
    done as 9 PSUM-accumulated matmuls of contraction 128 (last 32).
    Operands are "4-fold shifted" copies of q^T / k^T (Q4s / K4s) so each
    128-chunk of the contraction is a plain free-dim slice.
  - softmax row-wise (q on partitions): DVE max, ACT exp (+accum rowsum).
  - attn transposed per 128-chunk on the tensor engine, then
    out[q,d] = sum_c attnT[c].T @ vsum[c] accumulated in PSUM.
  - vsum via log-doubling shifted adds on DVE (all 4 heads at once).
All matmul operands fp16 (measured end-to-end rel err ~1.4e-3), PSUM f32.
"""

import numpy as np
from contextlib import ExitStack

S = 2048
E = 256
D = 32
WIN = 33
PAD = 16
K = S - WIN + 1  # 2016
NHPC = 4  # heads per core
SCALE = float(D) ** -0.5
NCORES = 8

_CACHE: dict = {}


def _build_nc(reps=1):
    import concourse.bass as bass
    import concourse.tile as tile
    from concourse import bacc, mybir

    fp16 = mybir.dt.float16
    f32 = mybir.dt.float32
    AF = mybir.ActivationFunctionType
    AX = mybir.AxisListType

    nc = bacc.Bacc("TRN2", target_bir_lowering=False, debug=False,
                   num_devices=NCORES)

    xT_d = nc.dram_tensor("xT", [E, S], f32, kind="ExternalInput").ap()
    wq_d = nc.dram_tensor("wq", [E, 128], f32, kind="ExternalInput").ap()
    wk_d = nc.dram_tensor("wk", [E, 128], f32, kind="ExternalInput").ap()
    wv_d = nc.dram_tensor("wv", [E, 128], f32, kind="ExternalInput").ap()
    bqs_d = nc.dram_tensor("bqs", [128, 1], f32, kind="ExternalInput").ap()
    bk_d = nc.dram_tensor("bk", [128, 1], f32, kind="ExternalInput").ap()
    bv_d = nc.dram_tensor("bv", [128, 1], f32, kind="ExternalInput").ap()
    out_d = nc.dram_tensor("out", [S, 128], f32, kind="ExternalOutput").ap()

    with tile.TileContext(nc) as tc, ExitStack() as ctx:
        const = ctx.enter_context(tc.tile_pool(name="const", bufs=1))
        persist = ctx.enter_context(tc.tile_pool(name="persist", bufs=1))

        # ---- load inputs (gpsimd DMAs cast f32 -> fp16 in flight) ----
        x16 = persist.tile([128, 2, S], fp16)  # x16[:, i, :] = xT[128i:128i+128, :]
        w16 = {}
        biases = {}
        for i in range(2):
            for sb in range(4):
                nc.gpsimd.dma_start(
                    out=x16[:, i, sb * 512:(sb + 1) * 512],
                    in_=xT_d[i * 128:(i + 1) * 128, sb * 512:(sb + 1) * 512])
        for name, wd in (("q", wq_d), ("k", wk_d), ("v", wv_d)):
            wt = const.tile([128, 2, 128], fp16, tag=f"w{name}")
            for i in range(2):
                nc.gpsimd.dma_start(out=wt[:, i, :], in_=wd[i * 128:(i + 1) * 128, :])
            w16[name] = wt
        for name, bd in (("q", bqs_d), ("k", bk_d), ("v", bv_d)):
            bt = const.tile([128, 1], f32, tag=f"b{name}")
            nc.gpsimd.dma_start(out=bt[:], in_=bd[:, :])
            biases[name] = bt

        # ---- projections: q^T,k^T,v^T [128, S] fp16 (q pre-scaled) ----
        qkv16 = {}
        with tc.tile_pool(name="pproj", bufs=2, space="PSUM") as pproj:
            for name in ("q", "k", "v"):
                dst = persist.tile([128, S], fp16, tag=f"{name}16T")
                qkv16[name] = dst
                sc = SCALE if name == "q" else 1.0
                for sb in range(4):
                    ps = pproj.tile([128, 512], f32, tag="pp")
                    nc.tensor.matmul(ps[:], lhsT=w16[name][:, 0, :],
                                     rhs=x16[:, 0, sb * 512:(sb + 1) * 512],
                                     start=True, stop=False)
                    nc.tensor.matmul(ps[:], lhsT=w16[name][:, 1, :],
                                     rhs=x16[:, 1, sb * 512:(sb + 1) * 512],
                                     start=False, stop=True)
                    nc.scalar.activation(out=dst[:, sb * 512:(sb + 1) * 512],
                                         in_=ps[:], func=AF.Identity,
                                         bias=biases[name], scale=sc)
        q16T, k16T, v16T = qkv16["q"], qkv16["k"], qkv16["v"]

        # ---- vsum^T[128, 2048] fp16 via log-doubling box filter (all heads).
        # Cols K..2048 zeroed so 128-wide XBAR transposes of the tail chunk
        # produce zero rows (which contribute nothing to the AV contraction).
        vsumT = persist.tile([128, S], fp16)
        nc.vector.memset(vsumT[:, K:S], 0.0)
        with tc.tile_pool(name="dbl", bufs=2) as dblp:
            t2 = dblp.tile([128, 2047], f32, tag="dbl")
            nc.vector.tensor_add(t2[:], v16T[:, 0:2047], v16T[:, 1:2048])
            prev, plen = t2, 2047
            for wshift in (2, 4, 8, 16):
                cur_len = plen - wshift
                cur = dblp.tile([128, 2045], f32, tag="dbl")
                nc.vector.tensor_add(cur[:, 0:cur_len], prev[:, 0:cur_len],
                                     prev[:, wshift:wshift + cur_len])
                prev, plen = cur, cur_len
            # width-32 sums now in prev[:, 0:2017]; add v[j+32] -> width 33
            nc.vector.tensor_add(vsumT[:, 0:K], prev[:, 0:K], v16T[:, 32:32 + K])

        # ---- pools for the main loop ----
        kq = ctx.enter_context(tc.tile_pool(name="kq", bufs=4))
        vs = ctx.enter_context(tc.tile_pool(name="vs", bufs=4))
        apool = ctx.enter_context(tc.tile_pool(name="apool", bufs=3))
        atpool = ctx.enter_context(tc.tile_pool(name="atpool", bufs=3))
        stats = ctx.enter_context(tc.tile_pool(name="stats", bufs=6))
        opool = ctx.enter_context(tc.tile_pool(name="opool", bufs=4))
        psum_sc = ctx.enter_context(tc.tile_pool(name="psc", bufs=6, space="PSUM"))
        psum_o = ctx.enter_context(tc.tile_pool(name="pso", bufs=2, space="PSUM"))

        for rep in range(reps):
         for h in range(NHPC):
            hp = 32 * h  # head's partition offset in q/k/v^T

            # vsum chunks [kx 128, d 32] via 2-byte XBAR DMA transpose
            vsum_sb = vs.tile([128, 16, D], fp16, tag="vsum")
            for ch in range(16):
                nc.sync.dma_start_transpose(
                    out=vsum_sb[:, ch, :],
                    in_=vsumT[hp:hp + 32, ch * 128:(ch + 1) * 128])

            # K4s[32r+d, j] = k^T[hp+d, j+r];  Q4s[32r+d, i] = q_pad^T[hp+d, i+r]
            K4s = kq.tile([128, S], fp16, tag="k4s")
            for r in range(4):
                half = (S - r) // 2
                nc.scalar.dma_start(out=K4s[32 * r:32 * r + 32, 0:half],
                                    in_=k16T[hp:hp + 32, r:r + half])
                nc.scalar.dma_start(out=K4s[32 * r:32 * r + 32, half:S - r],
                                    in_=k16T[hp:hp + 32, r + half:S])
            Q4s = kq.tile([128, S + 2 * PAD], fp16, tag="q4s")
            nc.vector.memset(Q4s[:, 0:PAD], 0.0)
            nc.vector.memset(Q4s[:, S:S + 2 * PAD], 0.0)
            for r in range(4):
                nc.gpsimd.dma_start(
                    out=Q4s[32 * r:32 * r + 32, PAD - r:PAD - r + 1024],
                    in_=q16T[hp:hp + 32, 0:1024])
                nc.gpsimd.dma_start(
                    out=Q4s[32 * r:32 * r + 32, PAD - r + 1024:PAD - r + S],
                    in_=q16T[hp:hp + 32, 1024:S])

            for t in range(16):
                q0 = t * 128
                blocks = []
                mx4 = stats.tile([128, 4], f32, tag="mx4")
                for blk in range(4):
                    c0 = blk * 504
                    ps = psum_sc.tile([128, 512], f32, tag="scores")
                    for a in range(8):
                        nc.tensor.matmul(
                            ps[:, 0:504],
                            lhsT=Q4s[:, q0 + 4 * a:q0 + 4 * a + 128],
                            rhs=K4s[:, 4 * a + c0:4 * a + c0 + 504],
                            start=(a == 0), stop=False)
                    nc.tensor.matmul(
                        ps[:, 0:504],
                        lhsT=Q4s[0:32, q0 + 32:q0 + 160],
                        rhs=K4s[0:32, 32 + c0:32 + c0 + 504],
                        start=False, stop=True)
                    nc.vector.tensor_reduce(out=mx4[:, blk:blk + 1],
                                            in_=ps[:, 0:504],
                                            op=mybir.AluOpType.max, axis=AX.X)
                    blocks.append(ps)
                negmx = stats.tile([128, 1], f32, tag="negmx")
                nc.vector.tensor_reduce(out=negmx[:], in_=mx4[:],
                                        op=mybir.AluOpType.max, axis=AX.X,
                                        negate=True)

                # attn padded to 2048 cols (zero tail) for 128-wide transposes
                attn = apool.tile([128, S], fp16, tag="attn")
                nc.vector.memset(attn[:, K:S], 0.0)
                racc = stats.tile([128, 4], f32, tag="racc")
                for blk in range(4):
                    nc.scalar.activation(
                        out=attn[:, blk * 504:(blk + 1) * 504],
                        in_=blocks[blk][:, 0:504],
                        func=AF.Exp, bias=negmx[:], scale=1.0,
                        accum_out=racc[:, blk:blk + 1])
                rsum = stats.tile([128, 1], f32, tag="rsum")
                nc.vector.tensor_reduce(out=rsum[:], in_=racc[:],
                                        op=mybir.AluOpType.add, axis=AX.X)
                rinv = stats.tile([128, 1], f32, tag="rinv")
                nc.vector.reciprocal(out=rinv[:], in_=rsum[:])

                attnT = atpool.tile([128, 16, 128], fp16, tag="attnT")
                for ch in range(16):
                    nc.sync.dma_start_transpose(
                        out=attnT[:, ch, :],
                        in_=attn[:, ch * 128:(ch + 1) * 128])

                po = psum_o.tile([128, D], f32, tag="pav")
                for ch in range(16):
                    nc.tensor.matmul(po[:], lhsT=attnT[:, ch, :],
                                     rhs=vsum_sb[:, ch, :],
                                     start=(ch == 0), stop=(ch == 15))
                ob = opool.tile([128, D], f32, tag="ob")
                nc.vector.tensor_scalar_mul(ob[:], po[:], rinv[:])
                nc.gpsimd.dma_start(out=out_d[q0:q0 + 128, hp:hp + 32], in_=ob[:])

    nc.compile()
    return nc


def _get_nc():
    if "nc" not in _CACHE:
        _CACHE["nc"] = _build_nc()
    return _CACHE["nc"]


def kernel(x, Wq, bq, Wk, bk, Wv, bv):
    from concourse.bass_utils import run_bass_kernel_spmd

    nc = _get_nc()
    x = np.asarray(x, dtype=np.float32)
    in_maps = []
    for c in range(NCORES):
        b, hg = c // 2, c % 2
        sl = slice(hg * 128, (hg + 1) * 128)
        in_maps.append({
            "xT": np.ascontiguousarray(x[b].T),
            "wq": np.ascontiguousarray(np.asarray(Wq, np.float32)[:, sl]),
            "wk": np.ascontiguousarray(np.asarray(Wk, np.float32)[:, sl]),
            "wv": np.ascontiguousarray(np.asarray(Wv, np.float32)[:, sl]),
            "bqs": np.ascontiguousarray(
                (np.asarray(bq, np.float32)[sl] * SCALE).reshape(128, 1)),
            "bk": np.ascontiguousarray(np.asarray(bk, np.float32)[sl].reshape(128, 1)),
            "bv": np.ascontiguousarray(np.asarray(bv, np.float32)[sl].reshape(128, 1)),
        })
    res = run_bass_kernel_spmd(nc, in_maps, list(range(NCORES)))
    out = np.empty((4, S, E), np.float32)
    for c in range(NCORES):
        b, hg = c // 2, c % 2
        out[b, :, hg * 128:(hg + 1) * 128] = res.results[c]["out"]
    return out


# revision 37
# speedup vs baseline: 8.9800x; 8.2076x over previous
"""Trainium2 Bass kernel for LocalSelfAttentionUnFold.

Reference math (B=4, S=2048, E=256, H=8, D=32, W=33, pad=16, K=S-W+1=2016):
  q,k,v = x @ W* + b*            -> [B,S,E] -> heads [B,H,S,D]
  scores[b,h,s,kx] = sum_{w,d} q_pad[b,h,s+w,d] * k[b,h,kx+w,d] * D^-0.5
  attn = softmax(scores, axis=kx)             # dense [S, K] matrix!
  out[b,h,s,d]  = sum_{kx} attn[s,kx] * vsum[kx,d],  vsum[kx] = sum_w v[kx+w]

Kernel strategy (per NeuronCore; 8 cores, core c handles batch b=c//2 and
head group hg=c%2, i.e. 4 heads = 128 embedding columns):
  - scores as a dense GEMM with the (w,d)-flattened contraction of 1056,
    done as 9 PSUM-accumulated matmuls of contraction 128 (last 32).
    Operands are "4-fold shifted" copies of q^T / k^T (Q4s / K4s) so each
    128-chunk of the contraction is a plain free-dim slice.
  - softmax row-wise (q on partitions): DVE max, ACT exp (+accum rowsum).
  - attn transposed per 128-chunk on the tensor engine, then
    out[q,d] = sum_c attnT[c].T @ vsum[c] accumulated in PSUM.
  - vsum via log-doubling shifted adds on DVE (all 4 heads at once).
All matmul operands fp16 (measured end-to-end rel err ~1.4e-3), PSUM f32.
"""

import numpy as np
from contextlib import ExitStack

S = 2048
E = 256
D = 32
WIN = 33
PAD = 16
K = S - WIN + 1  # 2016
NHPC = 4  # heads per core
SCALE = float(D) ** -0.5
NCORES = 8

_CACHE: dict = {}


def _build_nc(reps=1):
    import concourse.bass as bass
    import concourse.tile as tile
    from concourse import bacc, mybir

    fp16 = mybir.dt.float16
    f32 = mybir.dt.float32
    AF = mybir.ActivationFunctionType
    AX = mybir.AxisListType

    nc = bacc.Bacc("TRN2", target_bir_lowering=False, debug=False,
                   num_devices=NCORES)

    xT_d = nc.dram_tensor("xT", [E, S], f32, kind="ExternalInput").ap()
    wq_d = nc.dram_tensor("wq", [E, 128], f32, kind="ExternalInput").ap()
    wk_d = nc.dram_tensor("wk", [E, 128], f32, kind="ExternalInput").ap()
    wv_d = nc.dram_tensor("wv", [E, 128], f32, kind="ExternalInput").ap()
    bqs_d = nc.dram_tensor("bqs", [128, 1], f32, kind="ExternalInput").ap()
    bk_d = nc.dram_tensor("bk", [128, 1], f32, kind="ExternalInput").ap()
    bv_d = nc.dram_tensor("bv", [128, 1], f32, kind="ExternalInput").ap()
    bk4_d = nc.dram_tensor("bk4", [128, 1], f32, kind="ExternalInput").ap()
    bq4_d = nc.dram_tensor("bq4", [128, 1], f32, kind="ExternalInput").ap()
    out_d = nc.dram_tensor("out", [S, 128], f32, kind="ExternalOutput").ap()

    with tile.TileContext(nc) as tc, ExitStack() as ctx:
        const = ctx.enter_context(tc.tile_pool(name="const", bufs=1))
        persist = ctx.enter_context(tc.tile_pool(name="persist", bufs=1))

        # ---- load inputs (gpsimd DMAs cast f32 -> fp16 in flight) ----
        x16 = persist.tile([128, 2, S], fp16)  # x16[:, i, :] = xT[128i:128i+128, :]
        w16 = {}
        biases = {}
        for name, wd in (("k", wk_d), ("q", wq_d), ("v", wv_d)):
            wt = const.tile([128, 2, 128], fp16, tag=f"w{name}")
            wf = const.tile([128, 2, 128], f32, tag=f"wf{name}")
            for i in range(2):
                nc.scalar.dma_start(out=wf[:, i, :], in_=wd[i * 128:(i + 1) * 128, :])
                nc.vector.tensor_copy(out=wt[:, i, :], in_=wf[:, i, :])
            w16[name] = wt
        for name, bd in (("k", bk_d), ("q", bqs_d), ("v", bv_d),
                         ("k4", bk4_d), ("q4", bq4_d)):
            bt = const.tile([128, 1], f32, tag=f"b{name}")
            nc.scalar.dma_start(out=bt[:], in_=bd[:, :])
            biases[name] = bt
        for sb in range(4):
            for i in range(2):
                nc.gpsimd.dma_start(
                    out=x16[:, i, sb * 512:(sb + 1) * 512],
                    in_=xT_d[i * 128:(i + 1) * 128, sb * 512:(sb + 1) * 512])

        # ---- projections: q^T,k^T,v^T [128, S] fp16 (q pre-scaled) ----
        qkv16 = {}
        with tc.tile_pool(name="pproj", bufs=2, space="PSUM") as pproj:
            for name in ("k", "q", "v"):
                dst = persist.tile([128, S], fp16, tag=f"{name}16T")
                qkv16[name] = dst
                sc = SCALE if name == "q" else 1.0
                for sb in range(4):
                    ps = pproj.tile([128, 512], f32, tag="pp")
                    nc.tensor.matmul(ps[:], lhsT=w16[name][:, 0, :],
                                     rhs=x16[:, 0, sb * 512:(sb + 1) * 512],
                                     start=True, stop=False)
                    nc.tensor.matmul(ps[:], lhsT=w16[name][:, 1, :],
                                     rhs=x16[:, 1, sb * 512:(sb + 1) * 512],
                                     start=False, stop=True)
                    nc.scalar.activation(out=dst[:, sb * 512:(sb + 1) * 512],
                                         in_=ps[:], func=AF.Identity,
                                         bias=biases[name], scale=sc)
        q16T, k16T, v16T = qkv16["q"], qkv16["k"], qkv16["v"]

        # ---- head 0 K4s/Q4s built straight from projection matmuls:
        # psK[32r+d, n] = sum_E x16[E, s0+n] * W[E, d]  (col-tiled, 4 r-blocks)
        kq = ctx.enter_context(tc.tile_pool(name="kq", bufs=4))
        k4s0 = kq.tile([128, S], fp16, tag="k4s")
        q4s0 = kq.tile([128, S + 2 * PAD], fp16, tag="q4s")
        nc.vector.memset(q4s0[:, 0:PAD], 0.0)
        nc.vector.memset(q4s0[:, S:S + 2 * PAD], 0.0)
        with tc.tile_pool(name="pdir", bufs=2, space="PSUM") as pdir:
            for name, dst, b4 in (("k", k4s0, "k4"), ("q", q4s0, "q4")):
                qoff = 0 if name == "k" else PAD  # dst col of s=0 for r=0
                sc = SCALE if name == "q" else 1.0
                for sb in range(4):
                    ps = pdir.tile([128, 512], f32, tag="pd")
                    for r in range(4):
                        w = 512 if (sb < 3 or name == "q") else 512 - r
                        if name == "k":
                            rhs0, rhs1 = sb * 512 + r, sb * 512 + r + w
                        else:
                            rhs0, rhs1 = sb * 512, sb * 512 + w
                        for i in range(2):
                            nc.tensor.matmul(
                                ps[32 * r:32 * r + 32, 0:w],
                                lhsT=w16[name][:, i, 0:32],
                                rhs=x16[:, i, rhs0:rhs1],
                                start=(i == 0), stop=(i == 1),
                                tile_position=(0, 32 * r))
                    for r in range(4):
                        w = 512 if (sb < 3 or name == "q") else 512 - r
                        d0 = sb * 512 if name == "k" else PAD - r + sb * 512
                        nc.scalar.activation(
                            out=dst[32 * r:32 * r + 32, d0:d0 + w],
                            in_=ps[32 * r:32 * r + 32, 0:w],
                            func=AF.Identity, bias=biases[b4][32 * r:32 * r + 32],
                            scale=sc)

        # ---- vsum^T[128, 2048] fp16 via log-doubling box filter (all heads).
        # Cols K..2048 zeroed so 128-wide XBAR transposes of the tail chunk
        # produce zero rows (which contribute nothing to the AV contraction).
        vsumT = persist.tile([128, S], fp16)
        nc.vector.memset(vsumT[:, K:S], 0.0)
        with tc.tile_pool(name="dbl", bufs=2) as dblp:
            t2 = dblp.tile([128, 2047], f32, tag="dbl")
            nc.vector.tensor_add(t2[:], v16T[:, 0:2047], v16T[:, 1:2048])
            prev, plen = t2, 2047
            for wshift in (2, 4, 8, 16):
                cur_len = plen - wshift
                cur = dblp.tile([128, 2045], f32, tag="dbl")
                nc.vector.tensor_add(cur[:, 0:cur_len], prev[:, 0:cur_len],
                                     prev[:, wshift:wshift + cur_len])
                prev, plen = cur, cur_len
            # width-32 sums now in prev[:, 0:2017]; add v[j+32] -> width 33
            nc.vector.tensor_add(vsumT[:, 0:K], prev[:, 0:K], v16T[:, 32:32 + K])

        # ---- pools for the main loop ----
        vs = ctx.enter_context(tc.tile_pool(name="vs", bufs=2))
        apool = ctx.enter_context(tc.tile_pool(name="apool", bufs=5))
        atpool = ctx.enter_context(tc.tile_pool(name="atpool", bufs=3))
        stats = ctx.enter_context(tc.tile_pool(name="stats", bufs=6))
        opool = ctx.enter_context(tc.tile_pool(name="opool", bufs=4))
        psum_sc = ctx.enter_context(tc.tile_pool(name="psc", bufs=7, space="PSUM"))
        psum_o = ctx.enter_context(tc.tile_pool(name="pso", bufs=1, space="PSUM"))

        for rep in range(reps):
         for h in range(NHPC):
            hp = 32 * h  # head's partition offset in q/k/v^T

            # vsum chunks [kx 128, d 32] via 2-byte XBAR DMA transpose
            vsum_sb = vs.tile([128, 16, D], fp16, tag="vsum")
            for ch in range(16):
                nc.sync.dma_start_transpose(
                    out=vsum_sb[:, ch, :],
                    in_=vsumT[hp:hp + 32, ch * 128:(ch + 1) * 128])

            # K4s[32r+d, j] = k^T[hp+d, j+r];  Q4s[32r+d, i] = q_pad^T[hp+d, i+r]
            if h == 0 and rep == 0:
                K4s, Q4s = k4s0, q4s0
            else:
                K4s = kq.tile([128, S], fp16, tag="k4s")
                for r in range(4):
                    half = (S - r) // 2
                    nc.scalar.dma_start(out=K4s[32 * r:32 * r + 32, 0:half],
                                        in_=k16T[hp:hp + 32, r:r + half])
                for r in range(4):
                    half = (S - r) // 2
                    nc.scalar.dma_start(out=K4s[32 * r:32 * r + 32, half:S - r],
                                        in_=k16T[hp:hp + 32, r + half:S])
                Q4s = kq.tile([128, S + 2 * PAD], fp16, tag="q4s")
                nc.vector.memset(Q4s[:, 0:PAD], 0.0)
                nc.vector.memset(Q4s[:, S:S + 2 * PAD], 0.0)
                for r in range(4):
                    nc.gpsimd.dma_start(
                        out=Q4s[32 * r:32 * r + 32, PAD - r:PAD - r + 1024],
                        in_=q16T[hp:hp + 32, 0:1024])
                for r in range(4):
                    nc.gpsimd.dma_start(
                        out=Q4s[32 * r:32 * r + 32, PAD - r + 1024:PAD - r + S],
                        in_=q16T[hp:hp + 32, 1024:S])

            for t in range(16):
                q0 = t * 128
                blocks = []
                mx4 = stats.tile([128, 4], f32, tag="mx4")
                for blk in range(4):
                    c0 = blk * 504
                    ps = psum_sc.tile([128, 512], f32, tag="scores")
                    for a in range(8):
                        nc.tensor.matmul(
                            ps[:, 0:504],
                            lhsT=Q4s[:, q0 + 4 * a:q0 + 4 * a + 128],
                            rhs=K4s[:, 4 * a + c0:4 * a + c0 + 504],
                            start=(a == 0), stop=False)
                    blocks.append(ps)
                # w=32 tail for all 4 blocks: block blk uses shifted-copy
                # row-group blk (same data at col offset -blk), so the four
                # K=32 matmuls land on distinct 32-row groups + PSUM banks
                # and execute concurrently in the PE array.
                for blk in range(4):
                    c0 = blk * 504
                    nc.tensor.matmul(
                        blocks[blk][:, 0:504],
                        lhsT=Q4s[32 * blk:32 * blk + 32,
                                 q0 + 32 - blk:q0 + 160 - blk],
                        rhs=K4s[32 * blk:32 * blk + 32,
                                32 + c0 - blk:32 + c0 - blk + 504],
                        start=False, stop=True,
                        tile_position=(32 * blk, 0))
                for blk in range(4):
                    nc.vector.tensor_reduce(out=mx4[:, blk:blk + 1],
                                            in_=blocks[blk][:, 0:504],
                                            op=mybir.AluOpType.max, axis=AX.X)
                negmx = stats.tile([128, 1], f32, tag="negmx")
                nc.vector.tensor_reduce(out=negmx[:], in_=mx4[:],
                                        op=mybir.AluOpType.max, axis=AX.X,
                                        negate=True)

                # attn padded to 2048 cols (zero tail) for 128-wide transposes
                attn = apool.tile([128, S], fp16, tag="attn")
                nc.gpsimd.memset(attn[:, K:S], 0.0)
                racc = stats.tile([128, 4], f32, tag="racc")
                for blk in range(4):
                    nc.scalar.activation(
                        out=attn[:, blk * 504:(blk + 1) * 504],
                        in_=blocks[blk][:, 0:504],
                        func=AF.Exp, bias=negmx[:], scale=1.0,
                        accum_out=racc[:, blk:blk + 1])
                rsum = stats.tile([128, 1], f32, tag="rsum")
                nc.vector.tensor_reduce(out=rsum[:], in_=racc[:],
                                        op=mybir.AluOpType.add, axis=AX.X)
                rinv = stats.tile([128, 1], f32, tag="rinv")
                nc.vector.reciprocal(out=rinv[:], in_=rsum[:])

                attnT = atpool.tile([128, 16, 128], fp16, tag="attnT")
                for ch in range(16):
                    nc.sync.dma_start_transpose(
                        out=attnT[:, ch, 0:64],
                        in_=attn[0:64, ch * 128:(ch + 1) * 128])
                    nc.sync.dma_start_transpose(
                        out=attnT[:, ch, 64:128],
                        in_=attn[64:128, ch * 128:(ch + 1) * 128])

                po = psum_o.tile([128, D], f32, tag="pav")
                for ch in range(16):
                    nc.tensor.matmul(po[:], lhsT=attnT[:, ch, :],
                                     rhs=vsum_sb[:, ch, :],
                                     start=(ch == 0), stop=(ch == 15))
                ob = opool.tile([128, D], f32, tag="ob")
                nc.vector.tensor_scalar_mul(ob[:], po[:], rinv[:])
                nc.gpsimd.dma_start(out=out_d[q0:q0 + 128, hp:hp + 32], in_=ob[:])

    nc.compile()
    return nc


def _get_nc():
    if "nc" not in _CACHE:
        _CACHE["nc"] = _build_nc()
    return _CACHE["nc"]


def kernel(x, Wq, bq, Wk, bk, Wv, bv):
    from concourse.bass_utils import run_bass_kernel_spmd

    nc = _get_nc()
    x = np.asarray(x, dtype=np.float32)
    in_maps = []
    for c in range(NCORES):
        b, hg = c // 2, c % 2
        sl = slice(hg * 128, (hg + 1) * 128)
        in_maps.append({
            "xT": np.ascontiguousarray(x[b].T),
            "wq": np.ascontiguousarray(np.asarray(Wq, np.float32)[:, sl]),
            "wk": np.ascontiguousarray(np.asarray(Wk, np.float32)[:, sl]),
            "wv": np.ascontiguousarray(np.asarray(Wv, np.float32)[:, sl]),
            "bqs": np.ascontiguousarray(
                (np.asarray(bq, np.float32)[sl] * SCALE).reshape(128, 1)),
            "bk": np.ascontiguousarray(np.asarray(bk, np.float32)[sl].reshape(128, 1)),
            "bv": np.ascontiguousarray(np.asarray(bv, np.float32)[sl].reshape(128, 1)),
            "bk4": np.ascontiguousarray(np.tile(
                np.asarray(bk, np.float32)[sl][0:32], 4).reshape(128, 1)),
            "bq4": np.ascontiguousarray(np.tile(
                np.asarray(bq, np.float32)[sl][0:32] * SCALE, 4).reshape(128, 1)),
        })
    res = run_bass_kernel_spmd(nc, in_maps, list(range(NCORES)))
    out = np.empty((4, S, E), np.float32)
    for c in range(NCORES):
        b, hg = c // 2, c % 2
        out[b, :, hg * 128:(hg + 1) * 128] = res.results[c]["out"]
    return out


# revision 39
# speedup vs baseline: 9.0103x; 1.0034x over previous
"""Trainium2 Bass kernel for LocalSelfAttentionUnFold.

Reference math (B=4, S=2048, E=256, H=8, D=32, W=33, pad=16, K=S-W+1=2016):
  q,k,v = x @ W* + b*            -> [B,S,E] -> heads [B,H,S,D]
  scores[b,h,s,kx] = sum_{w,d} q_pad[b,h,s+w,d] * k[b,h,kx+w,d] * D^-0.5
  attn = softmax(scores, axis=kx)             # dense [S, K] matrix!
  out[b,h,s,d]  = sum_{kx} attn[s,kx] * vsum[kx,d],  vsum[kx] = sum_w v[kx+w]

Kernel strategy (per NeuronCore; 8 cores, core c handles batch b=c//2 and
head group hg=c%2, i.e. 4 heads = 128 embedding columns):
  - scores as a dense GEMM with the (w,d)-flattened contraction of 1056,
    done as 9 PSUM-accumulated matmuls of contraction 128 (last 32).
    Operands are "4-fold shifted" copies of q^T / k^T (Q4s / K4s) so each
    128-chunk of the contraction is a plain free-dim slice.
  - softmax row-wise (q on partitions): DVE max, ACT exp (+accum rowsum).
  - attn transposed per 128-chunk on the tensor engine, then
    out[q,d] = sum_c attnT[c].T @ vsum[c] accumulated in PSUM.
  - vsum via log-doubling shifted adds on DVE (all 4 heads at once).
All matmul operands fp16 (measured end-to-end rel err ~1.4e-3), PSUM f32.
"""

import numpy as np
from contextlib import ExitStack

S = 2048
E = 256
D = 32
WIN = 33
PAD = 16
K = S - WIN + 1  # 2016
NHPC = 4  # heads per core
SCALE = float(D) ** -0.5
NCORES = 8

_CACHE: dict = {}


def _build_nc(reps=1):
    import concourse.bass as bass
    import concourse.tile as tile
    from concourse import bacc, mybir

    fp16 = mybir.dt.float16
    f32 = mybir.dt.float32
    AF = mybir.ActivationFunctionType
    AX = mybir.AxisListType

    nc = bacc.Bacc("TRN2", target_bir_lowering=False, debug=False,
                   num_devices=NCORES)

    xT_d = nc.dram_tensor("xT", [E, S], f32, kind="ExternalInput").ap()
    wq_d = nc.dram_tensor("wq", [E, 128], f32, kind="ExternalInput").ap()
    wk_d = nc.dram_tensor("wk", [E, 128], f32, kind="ExternalInput").ap()
    wv_d = nc.dram_tensor("wv", [E, 128], f32, kind="ExternalInput").ap()
    bqs_d = nc.dram_tensor("bqs", [128, 1], f32, kind="ExternalInput").ap()
    bk_d = nc.dram_tensor("bk", [128, 1], f32, kind="ExternalInput").ap()
    bv_d = nc.dram_tensor("bv", [128, 1], f32, kind="ExternalInput").ap()
    bk4_d = nc.dram_tensor("bk4", [128, 1], f32, kind="ExternalInput").ap()
    bq4_d = nc.dram_tensor("bq4", [128, 1], f32, kind="ExternalInput").ap()
    out_d = nc.dram_tensor("out", [S, 128], f32, kind="ExternalOutput").ap()

    with tile.TileContext(nc) as tc, ExitStack() as ctx:
        const = ctx.enter_context(tc.tile_pool(name="const", bufs=1))
        persist = ctx.enter_context(tc.tile_pool(name="persist", bufs=1))

        # ---- load inputs (gpsimd DMAs cast f32 -> fp16 in flight) ----
        x16 = persist.tile([128, 2, S], fp16)  # x16[:, i, :] = xT[128i:128i+128, :]
        w16 = {}
        biases = {}
        for name, wd in (("k", wk_d), ("q", wq_d), ("v", wv_d)):
            wt = const.tile([128, 2, 128], fp16, tag=f"w{name}")
            wf = const.tile([128, 2, 128], f32, tag=f"wf{name}")
            for i in range(2):
                nc.scalar.dma_start(out=wf[:, i, :], in_=wd[i * 128:(i + 1) * 128, :])
                nc.vector.tensor_copy(out=wt[:, i, :], in_=wf[:, i, :])
            w16[name] = wt
        for name, bd in (("k", bk_d), ("q", bqs_d), ("v", bv_d),
                         ("k4", bk4_d), ("q4", bq4_d)):
            bt = const.tile([128, 1], f32, tag=f"b{name}")
            nc.scalar.dma_start(out=bt[:], in_=bd[:, :])
            biases[name] = bt
        for sb in range(4):
            for i in range(2):
                nc.gpsimd.dma_start(
                    out=x16[:, i, sb * 512:(sb + 1) * 512],
                    in_=xT_d[i * 128:(i + 1) * 128, sb * 512:(sb + 1) * 512])

        # ---- projections: q^T,k^T,v^T [128, S] fp16 (q pre-scaled) ----
        qkv16 = {}
        with tc.tile_pool(name="pproj", bufs=2, space="PSUM") as pproj:
            for name in ("k", "q", "v"):
                dst = persist.tile([128, S], fp16, tag=f"{name}16T")
                qkv16[name] = dst
                sc = SCALE if name == "q" else 1.0
                for sb in range(4):
                    ps = pproj.tile([128, 512], f32, tag="pp")
                    nc.tensor.matmul(ps[:], lhsT=w16[name][:, 0, :],
                                     rhs=x16[:, 0, sb * 512:(sb + 1) * 512],
                                     start=True, stop=False)
                    nc.tensor.matmul(ps[:], lhsT=w16[name][:, 1, :],
                                     rhs=x16[:, 1, sb * 512:(sb + 1) * 512],
                                     start=False, stop=True)
                    nc.scalar.activation(out=dst[:, sb * 512:(sb + 1) * 512],
                                         in_=ps[:], func=AF.Identity,
                                         bias=biases[name], scale=sc)
        q16T, k16T, v16T = qkv16["q"], qkv16["k"], qkv16["v"]

        # ---- head 0 K4s/Q4s built straight from projection matmuls:
        # psK[32r+d, n] = sum_E x16[E, s0+n] * W[E, d]  (col-tiled, 4 r-blocks)
        kq = ctx.enter_context(tc.tile_pool(name="kq", bufs=4))
        k4s0 = kq.tile([128, S], fp16, tag="k4s")
        q4s0 = kq.tile([128, S + 2 * PAD], fp16, tag="q4s")
        nc.vector.memset(q4s0[:, 0:PAD], 0.0)
        nc.vector.memset(q4s0[:, S:S + 2 * PAD], 0.0)
        with tc.tile_pool(name="pdir", bufs=2, space="PSUM") as pdir:
            for name, dst, b4 in (("k", k4s0, "k4"), ("q", q4s0, "q4")):
                qoff = 0 if name == "k" else PAD  # dst col of s=0 for r=0
                sc = SCALE if name == "q" else 1.0
                for sb in range(4):
                    ps = pdir.tile([128, 512], f32, tag="pd")
                    for r in range(4):
                        w = 512 if (sb < 3 or name == "q") else 512 - r
                        if name == "k":
                            rhs0, rhs1 = sb * 512 + r, sb * 512 + r + w
                        else:
                            rhs0, rhs1 = sb * 512, sb * 512 + w
                        for i in range(2):
                            nc.tensor.matmul(
                                ps[32 * r:32 * r + 32, 0:w],
                                lhsT=w16[name][:, i, 0:32],
                                rhs=x16[:, i, rhs0:rhs1],
                                start=(i == 0), stop=(i == 1),
                                tile_position=(0, 32 * r))
                    for r in range(4):
                        w = 512 if (sb < 3 or name == "q") else 512 - r
                        d0 = sb * 512 if name == "k" else PAD - r + sb * 512
                        if name == "k":
                            nc.vector.tensor_scalar_add(
                                dst[32 * r:32 * r + 32, d0:d0 + w],
                                ps[32 * r:32 * r + 32, 0:w],
                                biases[b4][32 * r:32 * r + 32])
                        else:
                            nc.scalar.activation(
                                out=dst[32 * r:32 * r + 32, d0:d0 + w],
                                in_=ps[32 * r:32 * r + 32, 0:w],
                                func=AF.Identity,
                                bias=biases[b4][32 * r:32 * r + 32],
                                scale=sc)

        # ---- vsum^T[128, 2048] fp16 via log-doubling box filter (all heads).
        # Cols K..2048 zeroed so 128-wide XBAR transposes of the tail chunk
        # produce zero rows (which contribute nothing to the AV contraction).
        vsumT = persist.tile([128, S], fp16)
        nc.vector.memset(vsumT[:, K:S], 0.0)
        with tc.tile_pool(name="dbl", bufs=2) as dblp:
            t2 = dblp.tile([128, 2047], f32, tag="dbl")
            nc.vector.tensor_add(t2[:], v16T[:, 0:2047], v16T[:, 1:2048])
            prev, plen = t2, 2047
            for wshift in (2, 4, 8, 16):
                cur_len = plen - wshift
                cur = dblp.tile([128, 2045], f32, tag="dbl")
                nc.vector.tensor_add(cur[:, 0:cur_len], prev[:, 0:cur_len],
                                     prev[:, wshift:wshift + cur_len])
                prev, plen = cur, cur_len
            # width-32 sums now in prev[:, 0:2017]; add v[j+32] -> width 33
            nc.vector.tensor_add(vsumT[:, 0:K], prev[:, 0:K], v16T[:, 32:32 + K])

        # ---- pools for the main loop ----
        vs = ctx.enter_context(tc.tile_pool(name="vs", bufs=2))
        apool = ctx.enter_context(tc.tile_pool(name="apool", bufs=5))
        atpool = ctx.enter_context(tc.tile_pool(name="atpool", bufs=3))
        stats = ctx.enter_context(tc.tile_pool(name="stats", bufs=6))
        opool = ctx.enter_context(tc.tile_pool(name="opool", bufs=4))
        psum_sc = ctx.enter_context(tc.tile_pool(name="psc", bufs=6, space="PSUM"))
        psum_o = ctx.enter_context(tc.tile_pool(name="pso", bufs=2, space="PSUM"))

        for rep in range(reps):
         for h in range(NHPC):
            hp = 32 * h  # head's partition offset in q/k/v^T

            # vsum chunks [kx 128, d 32] via 2-byte XBAR DMA transpose
            vsum_sb = vs.tile([128, 16, D], fp16, tag="vsum")
            for ch in range(16):
                nc.sync.dma_start_transpose(
                    out=vsum_sb[:, ch, :],
                    in_=vsumT[hp:hp + 32, ch * 128:(ch + 1) * 128])

            # K4s[32r+d, j] = k^T[hp+d, j+r];  Q4s[32r+d, i] = q_pad^T[hp+d, i+r]
            if h == 0 and rep == 0:
                K4s, Q4s = k4s0, q4s0
            else:
                K4s = kq.tile([128, S], fp16, tag="k4s")
                for r in range(4):
                    half = (S - r) // 2
                    nc.scalar.dma_start(out=K4s[32 * r:32 * r + 32, 0:half],
                                        in_=k16T[hp:hp + 32, r:r + half])
                for r in range(4):
                    half = (S - r) // 2
                    nc.scalar.dma_start(out=K4s[32 * r:32 * r + 32, half:S - r],
                                        in_=k16T[hp:hp + 32, r + half:S])
                Q4s = kq.tile([128, S + 2 * PAD], fp16, tag="q4s")
                nc.vector.memset(Q4s[:, 0:PAD], 0.0)
                nc.vector.memset(Q4s[:, S:S + 2 * PAD], 0.0)
                for r in range(4):
                    nc.gpsimd.dma_start(
                        out=Q4s[32 * r:32 * r + 32, PAD - r:PAD - r + 1024],
                        in_=q16T[hp:hp + 32, 0:1024])
                for r in range(4):
                    nc.gpsimd.dma_start(
                        out=Q4s[32 * r:32 * r + 32, PAD - r + 1024:PAD - r + S],
                        in_=q16T[hp:hp + 32, 1024:S])

            for t in range(16):
                q0 = t * 128
                blocks = []
                mx4 = stats.tile([128, 4], f32, tag="mx4")
                for blk in range(4):
                    c0 = blk * 504
                    ps = psum_sc.tile([128, 512], f32, tag="scores")
                    for a in range(8):
                        nc.tensor.matmul(
                            ps[:, 0:504],
                            lhsT=Q4s[:, q0 + 4 * a:q0 + 4 * a + 128],
                            rhs=K4s[:, 4 * a + c0:4 * a + c0 + 504],
                            start=(a == 0), stop=False)
                    blocks.append(ps)
                # w=32 tail for all 4 blocks: block blk uses shifted-copy
                # row-group blk (same data at col offset -blk), so the four
                # K=32 matmuls land on distinct 32-row groups + PSUM banks
                # and execute concurrently in the PE array.
                for blk in range(4):
                    c0 = blk * 504
                    nc.tensor.matmul(
                        blocks[blk][:, 0:504],
                        lhsT=Q4s[32 * blk:32 * blk + 32,
                                 q0 + 32 - blk:q0 + 160 - blk],
                        rhs=K4s[32 * blk:32 * blk + 32,
                                32 + c0 - blk:32 + c0 - blk + 504],
                        start=False, stop=True,
                        tile_position=(32 * blk, 0))
                for blk in range(4):
                    nc.vector.tensor_reduce(out=mx4[:, blk:blk + 1],
                                            in_=blocks[blk][:, 0:504],
                                            op=mybir.AluOpType.max, axis=AX.X)
                negmx = stats.tile([128, 1], f32, tag="negmx")
                nc.vector.tensor_reduce(out=negmx[:], in_=mx4[:],
                                        op=mybir.AluOpType.max, axis=AX.X,
                                        negate=True)

                # attn padded to 2048 cols (zero tail) for 128-wide transposes
                attn = apool.tile([128, S], fp16, tag="attn")
                nc.gpsimd.memset(attn[:, K:S], 0.0)
                racc = stats.tile([128, 4], f32, tag="racc")
                for blk in range(4):
                    nc.scalar.activation(
                        out=attn[:, blk * 504:(blk + 1) * 504],
                        in_=blocks[blk][:, 0:504],
                        func=AF.Exp, bias=negmx[:], scale=1.0,
                        accum_out=racc[:, blk:blk + 1])
                rsum = stats.tile([128, 1], f32, tag="rsum")
                nc.vector.tensor_reduce(out=rsum[:], in_=racc[:],
                                        op=mybir.AluOpType.add, axis=AX.X)
                rinv = stats.tile([128, 1], f32, tag="rinv")
                nc.vector.reciprocal(out=rinv[:], in_=rsum[:])

                attnT = atpool.tile([128, 16, 128], fp16, tag="attnT")
                for ch in range(16):
                    nc.sync.dma_start_transpose(
                        out=attnT[:, ch, 0:64],
                        in_=attn[0:64, ch * 128:(ch + 1) * 128])
                    nc.sync.dma_start_transpose(
                        out=attnT[:, ch, 64:128],
                        in_=attn[64:128, ch * 128:(ch + 1) * 128])

                po = psum_o.tile([128, D], f32, tag="pav")
                for ch in range(16):
                    nc.tensor.matmul(po[:], lhsT=attnT[:, ch, :],
                                     rhs=vsum_sb[:, ch, :],
                                     start=(ch == 0), stop=(ch == 15))
                ob = opool.tile([128, D], f32, tag="ob")
                nc.vector.tensor_scalar_mul(ob[:], po[:], rinv[:])
                nc.gpsimd.dma_start(out=out_d[q0:q0 + 128, hp:hp + 32], in_=ob[:])

    nc.compile()
    return nc


def _get_nc():
    if "nc" not in _CACHE:
        _CACHE["nc"] = _build_nc()
    return _CACHE["nc"]


def kernel(x, Wq, bq, Wk, bk, Wv, bv):
    from concourse.bass_utils import run_bass_kernel_spmd

    nc = _get_nc()
    x = np.asarray(x, dtype=np.float32)
    in_maps = []
    for c in range(NCORES):
        b, hg = c // 2, c % 2
        sl = slice(hg * 128, (hg + 1) * 128)
        in_maps.append({
            "xT": np.ascontiguousarray(x[b].T),
            "wq": np.ascontiguousarray(np.asarray(Wq, np.float32)[:, sl]),
            "wk": np.ascontiguousarray(np.asarray(Wk, np.float32)[:, sl]),
            "wv": np.ascontiguousarray(np.asarray(Wv, np.float32)[:, sl]),
            "bqs": np.ascontiguousarray(
                (np.asarray(bq, np.float32)[sl] * SCALE).reshape(128, 1)),
            "bk": np.ascontiguousarray(np.asarray(bk, np.float32)[sl].reshape(128, 1)),
            "bv": np.ascontiguousarray(np.asarray(bv, np.float32)[sl].reshape(128, 1)),
            "bk4": np.ascontiguousarray(np.tile(
                np.asarray(bk, np.float32)[sl][0:32], 4).reshape(128, 1)),
            "bq4": np.ascontiguousarray(np.tile(
                np.asarray(bq, np.float32)[sl][0:32] * SCALE, 4).reshape(128, 1)),
        })
    res = run_bass_kernel_spmd(nc, in_maps, list(range(NCORES)))
    out = np.empty((4, S, E), np.float32)
    for c in range(NCORES):
        b, hg = c // 2, c % 2
        out[b, :, hg * 128:(hg + 1) * 128] = res.results[c]["out"]
    return out


# revision 41
# speedup vs baseline: 9.0118x; 1.0002x over previous
"""Trainium2 Bass kernel for LocalSelfAttentionUnFold.

Reference math (B=4, S=2048, E=256, H=8, D=32, W=33, pad=16, K=S-W+1=2016):
  q,k,v = x @ W* + b*            -> [B,S,E] -> heads [B,H,S,D]
  scores[b,h,s,kx] = sum_{w,d} q_pad[b,h,s+w,d] * k[b,h,kx+w,d] * D^-0.5
  attn = softmax(scores, axis=kx)             # dense [S, K] matrix!
  out[b,h,s,d]  = sum_{kx} attn[s,kx] * vsum[kx,d],  vsum[kx] = sum_w v[kx+w]

Kernel strategy (per NeuronCore; 8 cores, core c handles batch b=c//2 and
head group hg=c%2, i.e. 4 heads = 128 embedding columns):
  - scores as a dense GEMM with the (w,d)-flattened contraction of 1056,
    done as 9 PSUM-accumulated matmuls of contraction 128 (last 32).
    Operands are "4-fold shifted" copies of q^T / k^T (Q4s / K4s) so each
    128-chunk of the contraction is a plain free-dim slice.
  - softmax row-wise (q on partitions): DVE max, ACT exp (+accum rowsum).
  - attn transposed per 128-chunk on the tensor engine, then
    out[q,d] = sum_c attnT[c].T @ vsum[c] accumulated in PSUM.
  - vsum via log-doubling shifted adds on DVE (all 4 heads at once).
All matmul operands fp16 (measured end-to-end rel err ~1.4e-3), PSUM f32.
"""

import numpy as np
from contextlib import ExitStack

S = 2048
E = 256
D = 32
WIN = 33
PAD = 16
K = S - WIN + 1  # 2016
NHPC = 4  # heads per core
SCALE = float(D) ** -0.5
NCORES = 8

_CACHE: dict = {}


def _build_nc(reps=1):
    import concourse.bass as bass
    import concourse.tile as tile
    from concourse import bacc, mybir

    fp16 = mybir.dt.float16
    f32 = mybir.dt.float32
    AF = mybir.ActivationFunctionType
    AX = mybir.AxisListType

    nc = bacc.Bacc("TRN2", target_bir_lowering=False, debug=False,
                   num_devices=NCORES)

    xT_d = nc.dram_tensor("xT", [E, S], f32, kind="ExternalInput").ap()
    wq_d = nc.dram_tensor("wq", [E, 128], f32, kind="ExternalInput").ap()
    wk_d = nc.dram_tensor("wk", [E, 128], f32, kind="ExternalInput").ap()
    wv_d = nc.dram_tensor("wv", [E, 128], f32, kind="ExternalInput").ap()
    bqs_d = nc.dram_tensor("bqs", [128, 1], f32, kind="ExternalInput").ap()
    bk_d = nc.dram_tensor("bk", [128, 1], f32, kind="ExternalInput").ap()
    bv_d = nc.dram_tensor("bv", [128, 1], f32, kind="ExternalInput").ap()
    bk4_d = nc.dram_tensor("bk4", [128, 1], f32, kind="ExternalInput").ap()
    bq4_d = nc.dram_tensor("bq4", [128, 1], f32, kind="ExternalInput").ap()
    out_d = nc.dram_tensor("out", [S, 128], f32, kind="ExternalOutput").ap()

    with tile.TileContext(nc) as tc, ExitStack() as ctx:
        const = ctx.enter_context(tc.tile_pool(name="const", bufs=1))
        persist = ctx.enter_context(tc.tile_pool(name="persist", bufs=1))

        # ---- load inputs (gpsimd DMAs cast f32 -> fp16 in flight) ----
        x16 = persist.tile([128, 2, S], fp16)  # x16[:, i, :] = xT[128i:128i+128, :]
        w16 = {}
        biases = {}
        for name, wd in (("k", wk_d), ("q", wq_d), ("v", wv_d)):
            wt = const.tile([128, 2, 128], fp16, tag=f"w{name}")
            wf = const.tile([128, 2, 128], f32, tag=f"wf{name}")
            for i in range(2):
                nc.scalar.dma_start(out=wf[:, i, :], in_=wd[i * 128:(i + 1) * 128, :])
                nc.vector.tensor_copy(out=wt[:, i, :], in_=wf[:, i, :])
            w16[name] = wt
        for name, bd in (("k", bk_d), ("q", bqs_d), ("v", bv_d),
                         ("k4", bk4_d), ("q4", bq4_d)):
            bt = const.tile([128, 1], f32, tag=f"b{name}")
            nc.scalar.dma_start(out=bt[:], in_=bd[:, :])
            biases[name] = bt
        for sb in range(4):
            for i in range(2):
                nc.gpsimd.dma_start(
                    out=x16[:, i, sb * 512:(sb + 1) * 512],
                    in_=xT_d[i * 128:(i + 1) * 128, sb * 512:(sb + 1) * 512])

        # ---- projections: q^T,k^T,v^T [128, S] fp16 (q pre-scaled) ----
        qkv16 = {}
        with tc.tile_pool(name="pproj", bufs=2, space="PSUM") as pproj:
            for name in ("k", "q", "v"):
                dst = persist.tile([128, S], fp16, tag=f"{name}16T")
                qkv16[name] = dst
                sc = SCALE if name == "q" else 1.0
                for sb in range(4):
                    ps = pproj.tile([128, 512], f32, tag="pp")
                    nc.tensor.matmul(ps[:], lhsT=w16[name][:, 0, :],
                                     rhs=x16[:, 0, sb * 512:(sb + 1) * 512],
                                     start=True, stop=False)
                    nc.tensor.matmul(ps[:], lhsT=w16[name][:, 1, :],
                                     rhs=x16[:, 1, sb * 512:(sb + 1) * 512],
                                     start=False, stop=True)
                    nc.scalar.activation(out=dst[:, sb * 512:(sb + 1) * 512],
                                         in_=ps[:], func=AF.Identity,
                                         bias=biases[name], scale=sc)
        q16T, k16T, v16T = qkv16["q"], qkv16["k"], qkv16["v"]

        # ---- head 0 K4s/Q4s built straight from projection matmuls:
        # psK[32r+d, n] = sum_E x16[E, s0+n] * W[E, d]  (col-tiled, 4 r-blocks)
        kq = ctx.enter_context(tc.tile_pool(name="kq", bufs=4))
        k4s0 = kq.tile([128, S], fp16, tag="k4s")
        q4s0 = kq.tile([128, S + 2 * PAD], fp16, tag="q4s")
        nc.vector.memset(q4s0[:, 0:PAD], 0.0)
        nc.vector.memset(q4s0[:, S:S + 2 * PAD], 0.0)
        with tc.tile_pool(name="pdir", bufs=2, space="PSUM") as pdir:
            for name, dst, b4 in (("k", k4s0, "k4"), ("q", q4s0, "q4")):
                qoff = 0 if name == "k" else PAD  # dst col of s=0 for r=0
                sc = SCALE if name == "q" else 1.0
                for sb in range(4):
                    ps = pdir.tile([128, 512], f32, tag="pd")
                    for r in range(4):
                        w = 512 if (sb < 3 or name == "q") else 512 - r
                        if name == "k":
                            rhs0, rhs1 = sb * 512 + r, sb * 512 + r + w
                        else:
                            rhs0, rhs1 = sb * 512, sb * 512 + w
                        for i in range(2):
                            nc.tensor.matmul(
                                ps[32 * r:32 * r + 32, 0:w],
                                lhsT=w16[name][:, i, 0:32],
                                rhs=x16[:, i, rhs0:rhs1],
                                start=(i == 0), stop=(i == 1),
                                tile_position=(0, 32 * r))
                    for r in range(4):
                        w = 512 if (sb < 3 or name == "q") else 512 - r
                        d0 = sb * 512 if name == "k" else PAD - r + sb * 512
                        if name == "k":
                            nc.vector.tensor_scalar_add(
                                dst[32 * r:32 * r + 32, d0:d0 + w],
                                ps[32 * r:32 * r + 32, 0:w],
                                biases[b4][32 * r:32 * r + 32])
                        else:
                            nc.scalar.activation(
                                out=dst[32 * r:32 * r + 32, d0:d0 + w],
                                in_=ps[32 * r:32 * r + 32, 0:w],
                                func=AF.Identity,
                                bias=biases[b4][32 * r:32 * r + 32],
                                scale=sc)

        # ---- vsum^T[128, 2048] fp16 via log-doubling box filter (all heads).
        # Cols K..2048 zeroed so 128-wide XBAR transposes of the tail chunk
        # produce zero rows (which contribute nothing to the AV contraction).
        vsumT = persist.tile([128, S], fp16)
        nc.vector.memset(vsumT[:, K:S], 0.0)
        with tc.tile_pool(name="dbl", bufs=2) as dblp:
            t2 = dblp.tile([128, 2047], f32, tag="dbl")
            nc.vector.tensor_add(t2[:], v16T[:, 0:2047], v16T[:, 1:2048])
            prev, plen = t2, 2047
            for wshift in (2, 4, 8, 16):
                cur_len = plen - wshift
                cur = dblp.tile([128, 2045], f32, tag="dbl")
                nc.vector.tensor_add(cur[:, 0:cur_len], prev[:, 0:cur_len],
                                     prev[:, wshift:wshift + cur_len])
                prev, plen = cur, cur_len
            # width-32 sums now in prev[:, 0:2017]; add v[j+32] -> width 33
            nc.vector.tensor_add(vsumT[:, 0:K], prev[:, 0:K], v16T[:, 32:32 + K])

        # ---- pools for the main loop ----
        vs = ctx.enter_context(tc.tile_pool(name="vs", bufs=2))
        apool = ctx.enter_context(tc.tile_pool(name="apool", bufs=5))
        atpool = ctx.enter_context(tc.tile_pool(name="atpool", bufs=3))
        stats = ctx.enter_context(tc.tile_pool(name="stats", bufs=6))
        opool = ctx.enter_context(tc.tile_pool(name="opool", bufs=4))
        psum_sc = ctx.enter_context(tc.tile_pool(name="psc", bufs=6, space="PSUM"))
        psum_o = ctx.enter_context(tc.tile_pool(name="pso", bufs=2, space="PSUM"))

        for rep in range(reps):
         for h in range(NHPC):
            hp = 32 * h  # head's partition offset in q/k/v^T

            # vsum chunks [kx 128, d 32] via 2-byte XBAR DMA transpose
            vsum_sb = vs.tile([128, 16, D], fp16, tag="vsum")
            for ch in range(16):
                nc.sync.dma_start_transpose(
                    out=vsum_sb[:, ch, :],
                    in_=vsumT[hp:hp + 32, ch * 128:(ch + 1) * 128])

            # K4s[32r+d, j] = k^T[hp+d, j+r];  Q4s[32r+d, i] = q_pad^T[hp+d, i+r]
            if h == 0 and rep == 0:
                K4s, Q4s = k4s0, q4s0
            else:
                K4s = kq.tile([128, S], fp16, tag="k4s")
                for r in range(4):
                    half = (S - r) // 2
                    nc.scalar.dma_start(out=K4s[32 * r:32 * r + 32, 0:half],
                                        in_=k16T[hp:hp + 32, r:r + half])
                for r in range(4):
                    half = (S - r) // 2
                    nc.scalar.dma_start(out=K4s[32 * r:32 * r + 32, half:S - r],
                                        in_=k16T[hp:hp + 32, r + half:S])
                Q4s = kq.tile([128, S + 2 * PAD], fp16, tag="q4s")
                nc.vector.memset(Q4s[:, 0:PAD], 0.0)
                nc.vector.memset(Q4s[:, S:S + 2 * PAD], 0.0)
                for r in range(4):
                    nc.gpsimd.dma_start(
                        out=Q4s[32 * r:32 * r + 32, PAD - r:PAD - r + 1024],
                        in_=q16T[hp:hp + 32, 0:1024])
                for r in range(4):
                    nc.gpsimd.dma_start(
                        out=Q4s[32 * r:32 * r + 32, PAD - r + 1024:PAD - r + S],
                        in_=q16T[hp:hp + 32, 1024:S])

            for t in range(16):
                q0 = t * 128
                blocks = []
                mx4 = stats.tile([128, 4], f32, tag="mx4")
                for blk in range(4):
                    c0 = blk * 504
                    ps = psum_sc.tile([128, 512], f32, tag="scores")
                    for a in range(8):
                        nc.tensor.matmul(
                            ps[:, 0:504],
                            lhsT=Q4s[:, q0 + 4 * a:q0 + 4 * a + 128],
                            rhs=K4s[:, 4 * a + c0:4 * a + c0 + 504],
                            start=(a == 0), stop=False)
                    blocks.append(ps)
                # w=32 tail for all 4 blocks: block blk uses shifted-copy
                # row-group blk (same data at col offset -blk), so the four
                # K=32 matmuls land on distinct 32-row groups + PSUM banks
                # and execute concurrently in the PE array.
                for blk in range(4):
                    c0 = blk * 504
                    nc.tensor.matmul(
                        blocks[blk][:, 0:504],
                        lhsT=Q4s[32 * blk:32 * blk + 32,
                                 q0 + 32 - blk:q0 + 160 - blk],
                        rhs=K4s[32 * blk:32 * blk + 32,
                                32 + c0 - blk:32 + c0 - blk + 504],
                        start=False, stop=True,
                        tile_position=(32 * blk, 0))
                for blk in range(4):
                    nc.vector.tensor_reduce(out=mx4[:, blk:blk + 1],
                                            in_=blocks[blk][:, 0:504],
                                            op=mybir.AluOpType.max, axis=AX.X)
                negmx = stats.tile([128, 1], f32, tag="negmx")
                nc.vector.tensor_reduce(out=negmx[:], in_=mx4[:],
                                        op=mybir.AluOpType.max, axis=AX.X,
                                        negate=True)

                # attn padded to 2048 cols (zero tail) for 128-wide transposes
                attn = apool.tile([128, S], fp16, tag="attn")
                nc.gpsimd.memset(attn[:, K:S], 0.0)
                racc = stats.tile([128, 4], f32, tag="racc")
                for blk in range(4):
                    nc.scalar.activation(
                        out=attn[:, blk * 504:(blk + 1) * 504],
                        in_=blocks[blk][:, 0:504],
                        func=AF.Exp, bias=negmx[:], scale=1.0,
                        accum_out=racc[:, blk:blk + 1])
                rsum = stats.tile([128, 1], f32, tag="rsum")
                nc.vector.tensor_reduce(out=rsum[:], in_=racc[:],
                                        op=mybir.AluOpType.add, axis=AX.X)
                rinv = stats.tile([128, 1], f32, tag="rinv")
                nc.vector.reciprocal(out=rinv[:], in_=rsum[:])

                attnT = atpool.tile([128, 16, 128], fp16, tag="attnT")
                for ch in range(16):
                    nc.sync.dma_start_transpose(
                        out=attnT[:, ch, 0:64],
                        in_=attn[0:64, ch * 128:(ch + 1) * 128])
                    nc.sync.dma_start_transpose(
                        out=attnT[:, ch, 64:128],
                        in_=attn[64:128, ch * 128:(ch + 1) * 128])

                po = psum_o.tile([128, D], f32, tag="pav")
                for ch in range(16):
                    nc.tensor.matmul(po[:], lhsT=attnT[:, ch, :],
                                     rhs=vsum_sb[:, ch, :],
                                     start=(ch == 0), stop=(ch == 15))
                ob = opool.tile([128, D], f32, tag="ob")
                nc.scalar.activation(out=ob[:], in_=po[:], func=AF.Identity,
                                     bias=0.0, scale=rinv[:])
                nc.gpsimd.dma_start(out=out_d[q0:q0 + 128, hp:hp + 32], in_=ob[:])

    nc.compile()
    return nc


def _get_nc():
    if "nc" not in _CACHE:
        _CACHE["nc"] = _build_nc()
    return _CACHE["nc"]


def kernel(x, Wq, bq, Wk, bk, Wv, bv):
    from concourse.bass_utils import run_bass_kernel_spmd

    nc = _get_nc()
    x = np.asarray(x, dtype=np.float32)
    in_maps = []
    for c in range(NCORES):
        b, hg = c // 2, c % 2
        sl = slice(hg * 128, (hg + 1) * 128)
        in_maps.append({
            "xT": np.ascontiguousarray(x[b].T),
            "wq": np.ascontiguousarray(np.asarray(Wq, np.float32)[:, sl]),
            "wk": np.ascontiguousarray(np.asarray(Wk, np.float32)[:, sl]),
            "wv": np.ascontiguousarray(np.asarray(Wv, np.float32)[:, sl]),
            "bqs": np.ascontiguousarray(
                (np.asarray(bq, np.float32)[sl] * SCALE).reshape(128, 1)),
            "bk": np.ascontiguousarray(np.asarray(bk, np.float32)[sl].reshape(128, 1)),
            "bv": np.ascontiguousarray(np.asarray(bv, np.float32)[sl].reshape(128, 1)),
            "bk4": np.ascontiguousarray(np.tile(
                np.asarray(bk, np.float32)[sl][0:32], 4).reshape(128, 1)),
            "bq4": np.ascontiguousarray(np.tile(
                np.asarray(bq, np.float32)[sl][0:32] * SCALE, 4).reshape(128, 1)),
        })
    res = run_bass_kernel_spmd(nc, in_maps, list(range(NCORES)))
    out = np.empty((4, S, E), np.float32)
    for c in range(NCORES):
        b, hg = c // 2, c % 2
        out[b, :, hg * 128:(hg + 1) * 128] = res.results[c]["out"]
    return out
